# revision 2
# baseline (speedup 1.0000x reference)
"""AxialClassifier kernel for 8 Trainium2 NeuronCores.

Strategy: pure data parallelism over the batch dimension (B=128 -> 16
images per core), parameters replicated on every core, per the
sharding hint.  One SPMD executable is compiled over a 1-D mesh of the
8 axon-tunneled NeuronCores; the batch axis of `x` (and of the output)
is sharded across the mesh, every parameter is replicated, and there
are no cross-core collectives in the graph.

All shapes are hardcoded; the module is self-contained.
"""

import numpy as np

# Hardcoded problem dimensions (nn_AxialClassifier_90855738179587).
B, S, D, H, E, L = 128, 48, 8, 2, 4, 8
HD = H * E
NUM_CLASSES = 7
N_CORES = 8

_PARAM_NAMES = [
    "enc_w", "enc_b", "pos_row", "pos_col",
    "Wq", "Wk", "Wv", "Wo", "bo", "cls_w", "cls_b",
]

_STATE = {}


def _forward(x, enc_w, enc_b, pos_row, pos_col, Wq, Wk, Wv, Wo, bo, cls_w, cls_b):
    import jax
    import jax.numpy as jnp

    def _self_attn(h, wq, wk, wv, wo, bo_):
        # h: (b, X, T, D); attend over axis T (second to last)
        b, X, T, _ = h.shape
        q = (h @ wq.T).reshape(b, X, T, H, E)
        k = (h @ wk.T).reshape(b, X, T, H, E)
        v = (h @ wv.T).reshape(b, X, T, H, E)
        dots = jnp.einsum("bxihe,bxjhe->bxhij", q, k) * (E**-0.5)
        attn = jax.nn.softmax(dots, axis=-1)
        o = jnp.einsum("bxhij,bxjhe->bxihe", attn, v).reshape(b, X, T, HD)
        return o @ wo.T + bo_

    h = jnp.transpose(x, (0, 2, 3, 1))
    h = jax.nn.relu(h @ enc_w.T + enc_b)
    h = h + pos_row[None, :, None, :] + pos_col[None, None, :, :]
    for l in range(L):
        ht = jnp.transpose(h, (0, 2, 1, 3))
        o_row = jnp.transpose(
            _self_attn(ht, Wq[l, 0], Wk[l, 0], Wv[l, 0], Wo[l, 0], bo[l, 0]),
            (0, 2, 1, 3),
        )
        o_col = _self_attn(h, Wq[l, 1], Wk[l, 1], Wv[l, 1], Wo[l, 1], bo[l, 1])
        h = jax.nn.relu(o_row + o_col)
    h = h.max(axis=-1)
    h = h.reshape(h.shape[0], -1)
    logits = h @ cls_w.T + cls_b
    return jax.nn.softmax(logits, axis=1)


def _setup():
    import jax
    from jax.sharding import Mesh, NamedSharding, PartitionSpec as P

    devs = jax.devices()[:N_CORES]
    mesh = Mesh(np.array(devs), axis_names=("dp",))
    batch_sh = NamedSharding(mesh, P("dp"))
    repl_sh = NamedSharding(mesh, P())

    in_shardings = [batch_sh] + [repl_sh] * len(_PARAM_NAMES)
    fwd = jax.jit(
        _forward,
        in_shardings=tuple(in_shardings),
        out_shardings=batch_sh,
    )
    _STATE["fwd"] = fwd
    _STATE["batch_sh"] = batch_sh
    _STATE["repl_sh"] = repl_sh
    return fwd


def kernel(**inputs) -> np.ndarray:
    import jax

    fwd = _STATE.get("fwd") or _setup()
    x = np.asarray(inputs["x"], dtype=np.float32)
    args = [jax.device_put(x, _STATE["batch_sh"])]
    for k in _PARAM_NAMES:
        args.append(
            jax.device_put(np.asarray(inputs[k], dtype=np.float32), _STATE["repl_sh"])
        )
    out = fwd(*args)
    return np.asarray(out).astype(np.float32)
